# revision 13
# baseline (speedup 1.0000x reference)
"""Trainium2 Bass kernel for nn_DenoiseNet (langevin point-cloud denoiser).

Strategy (8 NeuronCores, SPMD, zero inter-core communication):
  - Shard over B(2) x 4 contiguous N-chunks of 4096 points, each core padded
    with a 64-point halo on both sides (dependency cone grows 3 pts/step).
    Global-edge clipping handled exactly via per-core weight data (zeros on
    interior cores), so one program runs on all cores.
  - Feature-major layout [128 feat, (k, n) cols]. Sliding-window gather and
    scatter_add become free-dim shifted access patterns riding matmul PSUM
    accumulation.
  - fp8e4m3 DoubleRow matmuls (0.5 cyc/row) carry the heavy lifting:
      pass A: fp16 W0g.delta (exact state path) + one DR matmul adding the
              precomputed Gk context via hi/lo double-fp8 planes.
      pass B: DR pair (Wb1_hi | Wb1_lo*2^-4) - fp16-grade weights, half cost.
      pass C: DR pair (Wb2.h0 | Wb2.r1) - the residual h1 add never happens.
      scatter: DR pairs over (h0|r1) planes and (r2_k,r2_k+1) via a +k
              shifted layout that makes all 4 k-windows read at j=m+2.
    delta state stays fp16; its scatter update is an exact elementwise add.
  - Activations stored as fp8(2*h); biases pre-scaled 2x; the per-step
    langevin scale s_i folds into one ACT copy (scale=s_i/8) per chunk.
"""

import sys
import numpy as np
import ml_dtypes

for _p in ("/opt/trn_rl_repo",):
    if _p not in sys.path:
        sys.path.insert(0, _p)

import concourse.bass as bass
import concourse.bacc as bacc
import concourse.tile as tile
from concourse import mybir
from concourse.bass_utils import run_bass_kernel_spmd

# ---- problem constants (hardcoded per harness contract) ----
B, N, D = 2, 16384, 3
F = 128
K = 4
OFF = [-2, -1, 0, 1]
STEPS, S0, DECAY = 4, 0.2, 0.95
CHUNK, HALO, GW = 4096, 64, 2
NP = CHUNK + 2 * HALO          # 4224 local points
NB = NP + 2 * GW               # 4228 buffer cols (with guards)
R4 = K * NP                    # 16896 (k,n) columns
SR = NP + 16                   # r2 4D plane stride (16B-aligned for DR reads)
N_CORES = 8

f32 = mybir.dt.float32
f16 = mybir.dt.float16
f8 = mybir.dt.float8e4
AF = mybir.ActivationFunctionType
ALU = mybir.AluOpType
DRM = mybir.MatmulPerfMode.DoubleRow
E4 = ml_dtypes.float8_e4m3

_CH512 = [(c * 512, min(512, NP - c * 512)) for c in range((NP + 511) // 512)]
_CHNB = [(c * 512, min(512, NB - c * 512)) for c in range((NB + 511) // 512)]


def build_program(reps=1, loop_n=0):
    """Build the SPMD Bass/Tile program. Returns compiled Bacc module."""
    nc = bacc.Bacc("TRN2", target_bir_lowering=False, debug=False)

    def inp(name, shape, dt):
        return nc.dram_tensor(name, list(shape), dt, kind="ExternalInput").ap()

    d_pclT = inp("pclT", (4, NB), f16)
    d_delta0 = inp("delta0", (4, NB), f16)
    d_Wf1 = inp("Wf1", (3, F), f16)
    d_bf1 = inp("bf1", (F, 1), f32)
    d_WfW = inp("WfW", (F, F), f16)
    d_bg4 = inp("bg4", (F, 1), f32)
    d_W0g = inp("W0g", (3, F), f16)
    d_W0gn = inp("W0gn", (3, F), f16)
    d_IGk8 = inp("IGk8", (F, 2, F), f8)
    d_WB1 = inp("WB1hl8", (F, 2, F), f8)
    d_WB2 = inp("WB2d8", (F, 2, F), f8)
    d_WoD = inp("WoD8", (F, 2, 16), f8)
    d_WoS = inp("Wo8S", (F, 3), f8)
    d_cntW = inp("cntW16", (4, 3), f16)
    d_bb1 = inp("bb1x2", (F, 1), f32)
    d_bb2 = inp("bb2x2", (F, 1), f32)
    d_eLS = inp("eLS8", (F, 3), f8)
    d_eLnS = inp("eLnS8", (F, 3), f8)
    d_eRS = inp("eRS8", (F, 3), f8)
    d_eRnS = inp("eRnS8", (F, 3), f8)
    d_flagL = inp("flagL", (4, 1), f32)
    d_flagR = inp("flagR", (4, 1), f32)
    d_out = nc.dram_tensor("outT", [4, CHUNK], f16, kind="ExternalOutput").ap()

    from contextlib import ExitStack
    with tile.TileContext(nc) as tc, ExitStack() as ctx:
        cpool = ctx.enter_context(tc.tile_pool(name="const", bufs=1))
        tpool = ctx.enter_context(tc.tile_pool(name="tiny", bufs=2))
        hpool = ctx.enter_context(tc.tile_pool(name="h", bufs=4))
        psp = ctx.enter_context(tc.tile_pool(name="ps", bufs=6, space="PSUM"))
        pspp = ctx.enter_context(tc.tile_pool(name="psP", bufs=2, space="PSUM"))

        def load(dram, shape, dt, tag):
            t = cpool.tile(list(shape), dt, tag=tag)
            nc.sync.dma_start(t[:], dram[:])
            return t

        pclT = load(d_pclT, (4, NB), f16, "pclT")
        delta_a = load(d_delta0, (4, NB), f16, "delta_a")
        delta_b = load(d_delta0, (4, NB), f16, "delta_b")
        Wf1 = load(d_Wf1, (3, F), f16, "Wf1")
        bf1 = load(d_bf1, (F, 1), f32, "bf1")
        WfW = load(d_WfW, (F, F), f16, "WfW")
        bg4 = load(d_bg4, (F, 1), f32, "bg4")
        W0g = load(d_W0g, (3, F), f16, "W0g")
        W0gn = load(d_W0gn, (3, F), f16, "W0gn")
        IGk8 = load(d_IGk8, (F, 2, F), f8, "IGk8")
        WB1 = load(d_WB1, (F, 2, F), f8, "WB1")
        WB2 = load(d_WB2, (F, 2, F), f8, "WB2")
        WoD = load(d_WoD, (F, 2, 16), f8, "WoD")
        WoS = load(d_WoS, (F, 3), f8, "WoS")
        cntW = load(d_cntW, (4, 3), f16, "cntW")
        bb1 = load(d_bb1, (F, 1), f32, "bb1")
        bb2 = load(d_bb2, (F, 1), f32, "bb2")
        eLS = load(d_eLS, (F, 3), f8, "eLS")
        eLnS = load(d_eLnS, (F, 3), f8, "eLnS")
        eRS = load(d_eRS, (F, 3), f8, "eRS")
        eRnS = load(d_eRnS, (F, 3), f8, "eRnS")
        flagL = load(d_flagL, (4, 1), f32, "flagL")
        flagR = load(d_flagR, (4, 1), f32, "flagR")

        Gk8 = cpool.tile([F, 2, R4], f8, tag="Gk8")       # hi/lo planes
        HR = cpool.tile([F, 2, R4], f8, tag="HR")         # h0/r1 planes
        R2X = cpool.tile([F, 2, 2, SR], f8, tag="R2X")    # r2, +k shifted
        A0e4 = cpool.tile([F, NB], f16, tag="A0e4")       # 4*W0g.pcl_noisy
        G04 = cpool.tile([F, NP], f16, tag="G04")         # 4*G0

        # greedy engine balancer for elementwise work
        load_ns = {"ACT": 0.0, "DVE": 0.0, "GP": 0.0}

        def pick(cands):
            eng, cost, fn = min(cands, key=lambda c: load_ns[c[0]] + c[1])
            load_ns[eng] += cost
            fn()

        def relu_scale_op(dst, src, fd, scale):
            # dst = fp8(scale * relu(src)); no bias
            def on_act():
                nc.scalar.activation(dst, src, AF.Relu, bias=0.0, scale=scale)
            def on_dve():
                nc.vector.tensor_scalar(dst, src, float(scale), 0.0,
                                        ALU.mult, ALU.max)
            # GPSIMD cannot access PSUM (BIR verifier) - ACT/DVE only
            pick([("ACT", (fd + 212) * 0.833 + 16, on_act),
                  ("DVE", (fd + 120) * 1.042 + 15, on_dve)])

        def relu_bias_op(dst, src, fd, bias):
            # dst = fp8(relu(src + bias)); bias pre-scaled per-partition
            def on_act():
                nc.scalar.activation(dst, src, AF.Relu, bias=bias[:, :])
            def on_dve():
                nc.vector.tensor_scalar(dst, src, bias[:, :], 0.0,
                                        ALU.add, ALU.max)
            # GPSIMD cannot access PSUM (BIR verifier) - ACT/DVE only
            pick([("ACT", (fd + 212) * 0.833 + 16, on_act),
                  ("DVE", (fd + 120) * 1.042 + 15, on_dve)])

        def add16_op(dst, a, b, fd):
            # fp16 SBUF add (DVE 2x eligible)
            def on_dve():
                nc.vector.tensor_add(dst, a, b)
            def on_gp():
                nc.gpsimd.tensor_add(dst, a, b)
            pick([("DVE", (fd / 2 + 58) * 1.042 + 15, on_dve),
                  ("GP", fd * 0.833 / 0.42 + 95, on_gp)])

        # cone-garbage columns read before first write (see scatter shifts)
        nc.vector.memset(HR[:, :, 3 * NP - 1:3 * NP], 0.0)
        nc.vector.memset(R2X[:, 0, 0, NP:NP + 3], 0.0)
        nc.vector.memset(R2X[:, 0, 1, NP + 1:NP + 3], 0.0)
        nc.vector.memset(R2X[:, 1, 1, 0:3], 0.0)

        # ---------------- preamble: A0e4, G04, Gk8 ----------------
        for c0, fd in _CHNB:
            ps = psp.tile([F, 512], f32, tag="ps")
            nc.tensor.matmul(ps[:, :fd], W0g[:, :], pclT[0:3, c0:c0 + fd],
                             start=True, stop=True)
            def mk_a0(c0=c0, fd=fd, ps=ps):
                def on_act():
                    nc.scalar.activation(A0e4[:, c0:c0 + fd], ps[:, :fd],
                                         AF.Copy, bias=0.0, scale=4.0)
                def on_dve():
                    nc.vector.tensor_scalar_mul(A0e4[:, c0:c0 + fd], ps[:, :fd],
                                                4.0)
                pick([("ACT", (fd + 212) * 0.833 + 16, on_act),
                      ("DVE", (fd + 120) * 1.042 + 15, on_dve)])
            mk_a0()
        for c0, fd in _CH512:
            ps = psp.tile([F, 512], f32, tag="ps")
            nc.tensor.matmul(ps[:, :fd], Wf1[:, :], pclT[0:3, GW + c0:GW + c0 + fd],
                             start=True, stop=True)
            hf = hpool.tile([F, 512], f16, tag="hf")
            nc.scalar.activation(hf[:, :fd], ps[:, :fd], AF.Relu, bias=bf1[:, :])
            ps2 = psp.tile([F, 512], f32, tag="ps")
            nc.tensor.matmul(ps2[:, :fd], WfW[:, :], hf[:, :fd], start=True, stop=False)
            nc.tensor.matmul(ps2[:, :fd], W0gn[:, :], pclT[0:3, GW + c0:GW + c0 + fd],
                             start=False, stop=True)
            nc.scalar.activation(G04[:, c0:c0 + fd], ps2[:, :fd], AF.Identity,
                                 bias=bg4[:, :], scale=4.0)
        # Gk8 hi/lo: s16 = G04 + A0e4[shifted]; hi = fp8(s16);
        # lo = fp8(16*(s16 - hi))
        for k in range(K):
            for c0, fd in _CH512:
                col = k * NP + c0
                s16 = hpool.tile([F, 512], f16, tag="s16")
                add16_op(s16[:, :fd], G04[:, c0:c0 + fd],
                         A0e4[:, GW + OFF[k] + c0:GW + OFF[k] + c0 + fd], fd)
                def mk_hi(col=col, fd=fd, s16=s16):
                    def on_act():
                        nc.scalar.activation(Gk8[:, 0, col:col + fd], s16[:, :fd],
                                             AF.Copy)
                    def on_dve():
                        nc.vector.tensor_copy(Gk8[:, 0, col:col + fd], s16[:, :fd])
                    def on_gp():
                        nc.gpsimd.tensor_copy(Gk8[:, 0, col:col + fd], s16[:, :fd])
                    pick([("ACT", (fd + 212) * 0.833 + 16, on_act),
                          ("DVE", (fd + 58) * 1.042 + 15, on_dve),
                          ("GP", fd * 0.833 / 0.6 + 95, on_gp)])
                mk_hi()
                d16 = hpool.tile([F, 512], f16, tag="d16")
                def mk_sub(col=col, fd=fd, s16=s16, d16=d16):
                    def on_dve():
                        nc.vector.tensor_sub(d16[:, :fd], s16[:, :fd],
                                             Gk8[:, 0, col:col + fd])
                    def on_gp():
                        nc.gpsimd.tensor_sub(d16[:, :fd], s16[:, :fd],
                                             Gk8[:, 0, col:col + fd])
                    pick([("DVE", (fd + 58) * 1.042 + 15, on_dve),
                          ("GP", fd * 0.833 / 0.42 + 95, on_gp)])
                mk_sub()
                nc.scalar.activation(Gk8[:, 1, col:col + fd], d16[:, :fd],
                                     AF.Copy, bias=0.0, scale=16.0)
                load_ns["ACT"] += (fd + 212) * 0.833 + 16

        # ---------------- langevin steps ----------------
        def emit_rep(final_rep):
            for step in range(STEPS):
                d_in = delta_a if step % 2 == 0 else delta_b
                d_out_t = delta_b if step % 2 == 0 else delta_a
                final = (step == STEPS - 1) and final_rep
                s_i = S0 * DECAY ** step

                def emit_passA(cb):
                    c0, fd = _CH512[cb]
                    for k in range(K):
                        col = k * NP + c0
                        ps = psp.tile([F, 512], f32, tag="ps")
                        nc.tensor.matmul(
                            ps[:, :fd], W0g[:, :],
                            d_in[0:3, GW + OFF[k] + c0:GW + OFF[k] + c0 + fd],
                            start=True, stop=False)
                        nc.tensor.matmul(ps[:, :fd], IGk8[:, :, :],
                                         Gk8[:, :, col:col + fd],
                                         start=False, stop=True, perf_mode=DRM)
                        relu_scale_op(HR[:, 0, col:col + fd], ps[:, :fd], fd, 2.0)

                def emit_passB(cb):
                    c0, fd = _CH512[cb]
                    for k in range(K):
                        col = k * NP + c0
                        ps = psp.tile([F, 512], f32, tag="ps")
                        h0b = HR[:, 0:1, col:col + fd].to_broadcast((F, 2, fd))
                        nc.tensor.matmul(ps[:, :fd], WB1[:, :, :], h0b,
                                         start=True, stop=True, perf_mode=DRM)
                        relu_bias_op(HR[:, 1, col:col + fd], ps[:, :fd], fd, bb1)

                def emit_passC(cb):
                    c0, fd = _CH512[cb]
                    for k in range(K):
                        col = k * NP + c0
                        ps = psp.tile([F, 512], f32, tag="ps")
                        nc.tensor.matmul(ps[:, :fd], WB2[:, :, :],
                                         HR[:, :, col:col + fd],
                                         start=True, stop=True, perf_mode=DRM)
                        relu_bias_op(R2X[:, k // 2, k % 2, k + c0:k + c0 + fd],
                                     ps[:, :fd], fd, bb2)

                def mirror_fix(flag, src_l, dst_ls):
                    # mirror guards at global edges (flag=0 -> no-op interior)
                    for dst_l in dst_ls:
                        t = tpool.tile([4, 1], f16, tag="mir")
                        nc.vector.tensor_sub(t[0:3, :],
                                             d_out_t[0:3, GW + src_l:GW + src_l + 1],
                                             d_out_t[0:3, GW + dst_l:GW + dst_l + 1])
                        nc.vector.tensor_scalar_mul(t[0:3, :], t[0:3, :], flag[0:3, :])
                        nc.vector.tensor_add(d_out_t[0:3, GW + dst_l:GW + dst_l + 1],
                                             d_out_t[0:3, GW + dst_l:GW + dst_l + 1],
                                             t[0:3, :])

                def corr(pcol, Sw, col):
                    # += Sw.(h0+r1+r2)[col]  (edge corrections, plain fp8)
                    k, n = col // NP, col % NP
                    for plane in range(2):
                        nc.tensor.matmul(pcol, Sw[:, :],
                                         HR[:, plane, col:col + 1],
                                         start=False, stop=False)
                    nc.tensor.matmul(pcol, Sw[:, :],
                                     R2X[:, k // 2, k % 2, k + n:k + n + 1],
                                     start=False, stop=False)

                def emit_scatter(cb):
                    c0, fd = _CH512[cb]
                    ps = pspp.tile([16, 512], f32, tag="psP")
                    for k in (0, 2):    # even k: DR-aligned (h0|r1) pairs
                        st = k * NP + c0 - OFF[k]
                        nc.tensor.matmul(ps[0:16, :fd], WoD[:, :, :],
                                         HR[:, :, st:st + fd],
                                         start=(k == 0), stop=False, perf_mode=DRM)
                    for k in (1, 3):    # odd k: 2B-misaligned, plain fp8
                        st = k * NP + c0 - OFF[k]
                        for plane in range(2):
                            nc.tensor.matmul(ps[0:3, :fd], WoS[:, :],
                                             HR[:, plane, st:st + fd],
                                             start=False, stop=False)
                    for p in range(2):
                        nc.tensor.matmul(ps[0:16, :fd], WoD[:, :, :],
                                         R2X[:, p, :, c0 + 2:c0 + 2 + fd],
                                         start=False, stop=False, perf_mode=DRM)
                    if cb == 0:
                        pcol = ps[0:3, HALO:HALO + 1]
                        for col in (HALO, HALO + 1, NP + HALO):
                            corr(pcol, eLS, col)
                        corr(pcol, eLnS, 3 * NP + HALO - 1)
                    if cb == len(_CH512) - 1:
                        lN = HALO + CHUNK - 1
                        pN = ps[0:3, lN - c0:lN - c0 + 1]
                        corr(pN, eRS, 3 * NP + lN)
                        for col in (lN + 2, NP + lN + 1):
                            corr(pN, eRnS, col)
                        corr(ps[0:3, lN - 1 - c0:lN - c0], eRnS, lN + 1)
                    nc.tensor.matmul(ps[0:3, :fd], cntW[:, :],
                                     d_in[0:4, GW + c0:GW + c0 + fd],
                                     start=False, stop=True)
                    t16 = hpool.tile([4, 512], f16, tag="t16")
                    nc.scalar.activation(t16[0:3, :fd], ps[0:3, :fd], AF.Copy,
                                         bias=0.0, scale=s_i / 8.0)
                    load_ns["ACT"] += (fd + 212) * 0.833 + 16
                    add16_op(d_out_t[0:3, GW + c0:GW + c0 + fd],
                             d_in[0:3, GW + c0:GW + c0 + fd], t16[0:3, :fd], fd)

                nblk = len(_CH512)
                for cb in range(nblk + 5):
                    if cb < nblk:
                        emit_passA(cb)
                    if 0 <= cb - 2 < nblk:
                        emit_passB(cb - 2)
                    if 0 <= cb - 3 < nblk:
                        emit_passC(cb - 3)
                    if 0 <= cb - 5 < nblk:
                        emit_scatter(cb - 5)

                if final:
                    nc.sync.dma_start(
                        d_out[:, :], d_out_t[0:4, GW + HALO:GW + HALO + CHUNK])
                else:
                    mirror_fix(flagL, HALO, (HALO - 2, HALO - 1))
                    mirror_fix(flagR, HALO + CHUNK - 1, (HALO + CHUNK,))

        if loop_n:
            with tc.For_i(0, loop_n, 1):
                emit_rep(False)
            emit_rep(True)
        else:
            for rep in range(reps):
                emit_rep(rep == reps - 1)

    nc.compile()
    return nc


def _q8(x, scale=1.0):
    return (np.asarray(x, np.float32) * scale).astype(E4)


def host_prep(inputs):
    """Slice/transpose/pad inputs per core; build weight constants."""
    pcl = np.asarray(inputs["pcl_noisy"], np.float32)
    Wf1 = np.asarray(inputs["Wf1"], np.float32)
    bf1 = np.asarray(inputs["bf1"], np.float32)
    Wf2 = np.asarray(inputs["Wf2"], np.float32)
    bf2 = np.asarray(inputs["bf2"], np.float32)
    W0 = np.asarray(inputs["W0"], np.float32)
    b0 = np.asarray(inputs["b0"], np.float32)
    Wb = np.asarray(inputs["Wb"], np.float32)
    bb = np.asarray(inputs["bb"], np.float32)
    Wo = np.asarray(inputs["Wo"], np.float32)
    bo = np.asarray(inputs["bo"], np.float32)

    W0g = W0[:3]
    WfW = Wf2 @ W0[3:]
    bg = bf2 @ W0[3:] + b0
    offs = np.arange(-(K - 1) // 2, (K - 1) // 2 + 1)
    nbr = np.clip(np.arange(N)[:, None] + offs, 0, N - 1).reshape(-1)
    c_global = np.bincount(nbr, minlength=N).astype(np.float32)

    hf = np.float16
    eye = np.eye(F, dtype=np.float32)
    IGk8 = np.stack([_q8(eye * 0.25), _q8(eye / 64.0)], axis=1)
    # split-half pair: sub0 holds rows 0:64, sub1 rows 64:128 (rhs is the
    # same h0 broadcast twice, so the halves sum to the full product)
    wb1 = _q8(Wb[0])
    wb1t = np.zeros_like(wb1)
    wb1b = np.zeros_like(wb1)
    wb1t[0:64] = wb1[0:64]
    wb1b[64:128] = wb1[64:128]
    WB1hl8 = np.stack([wb1t, wb1b], axis=1)
    wb2 = _q8(Wb[1])
    WB2d8 = np.stack([wb2, wb2], axis=1)
    wo4 = _q8(4.0 * Wo)
    wo4p = np.zeros((F, 16), E4)
    wo4p[:, 0:3] = wo4
    WoD8 = np.stack([wo4p, wo4p], axis=1)
    cntW16 = np.zeros((4, 3), hf)
    cntW16[3] = (8.0 * bo).astype(hf)

    zS = np.zeros((F, 3), E4)
    shared = {
        "Wf1": Wf1.astype(hf), "bf1": bf1.reshape(F, 1).astype(np.float32),
        "WfW": WfW.astype(hf),
        "bg4": (4.0 * bg).reshape(F, 1).astype(np.float32),
        "W0g": W0g.astype(hf), "W0gn": (-W0g).astype(hf),
        "IGk8": IGk8, "WB1hl8": WB1hl8, "WB2d8": WB2d8, "WoD8": WoD8,
        "Wo8S": wo4, "cntW16": cntW16,
        "bb1x2": (2.0 * bb[0]).reshape(F, 1).astype(np.float32),
        "bb2x2": (2.0 * bb[1]).reshape(F, 1).astype(np.float32),
    }
    in_maps = []
    for core in range(N_CORES):
        b, ch = core // 4, core % 4
        g0 = ch * CHUNK - HALO
        idx = np.clip(np.arange(g0 - GW, g0 + NP + GW), 0, N - 1)
        pclT = np.empty((4, NB), hf)
        pclT[0:3] = pcl[b, idx].T.astype(hf)
        pclT[3] = 0.0
        delta0 = np.zeros((4, NB), hf)
        delta0[3, GW:GW + NP] = c_global[np.clip(np.arange(g0, g0 + NP), 0, N - 1)]
        isL, isR = ch == 0, ch == 3
        m = dict(shared)
        m["pclT"] = pclT
        m["delta0"] = delta0
        m["eLS8"] = wo4 if isL else zS
        m["eLnS8"] = _q8(-4.0 * Wo) if isL else zS
        m["eRS8"] = wo4 if isR else zS
        m["eRnS8"] = _q8(-4.0 * Wo) if isR else zS
        m["flagL"] = np.full((4, 1), 1.0 if isL else 0.0, np.float32)
        m["flagR"] = np.full((4, 1), 1.0 if isR else 0.0, np.float32)
        in_maps.append(m)
    return in_maps


_CACHED = {}


def _get_program(reps=1):
    if reps not in _CACHED:
        _CACHED[reps] = build_program(reps)
    return _CACHED[reps]


def kernel(**inputs):
    nc = _get_program(1)
    in_maps = host_prep(inputs)
    res = run_bass_kernel_spmd(nc, in_maps, list(range(N_CORES)))
    pcl = np.asarray(inputs["pcl_noisy"], np.float32)
    out = np.empty((B, N, D), np.float32)
    for core in range(N_CORES):
        b, ch = core // 4, core % 4
        sl = slice(ch * CHUNK, (ch + 1) * CHUNK)
        out[b, sl] = pcl[b, sl] + res.results[core]["outT"][0:3].T.astype(np.float32)
    return out
